# revision 6
# baseline (speedup 1.0000x reference)
"""Multi-head self-attention (B=2, S=2048, D=1024, H=16, causal+padding mask)
on 8 Trainium2 NeuronCores via Bass/Tile, SPMD.

Sharding: core c -> batch b = c//4, query residue r = c%4. Each core computes
the full K/V projections for its batch (duplicated across the 4 cores of a
batch -- cheaper than any cross-core collective at this size) and attention +
output projection for the strided query set q = 4j + r, j = 0..511. Strided
(rather than blocked) query assignment keeps the causal loop structure
identical on every core, which SPMD requires; per-core differences (mask
values, packed activations) travel as data.

Dataflow is fully transposed so no on-chip transposes are needed:
  QT[dh, q]   = (Wq x^T)/8 + bq/8          lhsT = Wq^T chunks, rhs = xq^T
  KT[dh, k]   = Wk x^T + bk
  V [k, dh]   = x Wv^T  (+ ones column)    lhsT = x^T chunks,  rhs = Wv^T
  ST[k, q]    = KT_h^T QT_h  (per head)
  E           = exp(ST + padmask_bias) * causal01
  OT'[dh+1,q] = V_aug^T E   (row 64 = softmax denominators r[q])
  AT[dh, q]   = OT' * (1/r)                (broadcast 1/r via rank-1 matmul)
  YT[n, q]    = Wo_arr^T AT + (bo + Wo bv) lhsT = Wo^T chunks, rhs = AT
Matmuls run in float32r (full-rate fp32 with reduced mantissa, ~1e-4 rel
error). Softmax skips max-subtraction: scores are bounded (|S| < ~5) so exp
is safe, and masked lanes get -1e4 added pre-exp which underflows to exactly
0 after exp.
"""

import sys

if "/opt/trn_rl_repo" not in sys.path:
    sys.path.insert(0, "/opt/trn_rl_repo")

import numpy as np

B, S, D, H, HD = 2, 2048, 1024, 16, 64
N_CORES = 8
JQ = S // 4          # 512 packed queries per core
MC = D // 128        # 8 contraction chunks of 128
NKT = S // 128       # 16 key tiles
JB_N = 256           # packed query block (matmul N)

_CACHE = {}


def _split_waits(nc, mybir):
    """This walrus build accepts only one sync-wait per instruction; move
    extra waits onto NOPs inserted just before, on the same engine."""
    n_new = 0
    for f in nc.m.functions:
        for blk in f.blocks:
            out = []
            for inst in blk.instructions:
                si = inst.sync_info
                if si is not None and si.on_wait is not None and len(si.on_wait) > 1:
                    waits = list(si.on_wait)
                    for w in waits[:-1]:
                        n_new += 1
                        out.append(mybir.InstNoOp(
                            name=f"I-waitsplit-{n_new}",
                            engine=inst.engine,
                            ins=[], outs=[],
                            sync_info=mybir.SyncInfo(on_wait=[w], on_update=[]),
                        ))
                    inst.sync_info = mybir.SyncInfo(
                        on_wait=[waits[-1]], on_update=list(si.on_update or []))
                out.append(inst)
            blk.instructions[:] = out
    return n_new


def _build():
    import concourse.bass as bass
    import concourse.mybir as mybir
    import concourse.tile as tile

    f32 = mybir.dt.float32
    f32r = mybir.dt.float32r
    EXP = mybir.ActivationFunctionType.Exp
    IDENT = mybir.ActivationFunctionType.Identity
    COPY = mybir.ActivationFunctionType.Copy

    nc = bass.Bass()
    xT = nc.declare_dram_parameter("xT", [D, S], f32r, isOutput=False)
    xqT = nc.declare_dram_parameter("xqT", [D, JQ], f32r, isOutput=False)
    wqT = nc.declare_dram_parameter("wqT", [D, D], f32r, isOutput=False)
    wkT = nc.declare_dram_parameter("wkT", [D, D], f32r, isOutput=False)
    wvT = nc.declare_dram_parameter("wvT", [D, D], f32r, isOutput=False)
    woT = nc.declare_dram_parameter("woT", [D, D], f32r, isOutput=False)
    bq8 = nc.declare_dram_parameter("bq8", [D], f32, isOutput=False)
    bkv = nc.declare_dram_parameter("bk", [D], f32, isOutput=False)
    obias = nc.declare_dram_parameter("obias", [D], f32, isOutput=False)
    pmb = nc.declare_dram_parameter("pmb", [S], f32, isOutput=False)
    cmask = nc.declare_dram_parameter("cmask", [8, 128, JB_N], f32r, isOutput=False)
    onesc = nc.declare_dram_parameter("onesc", [1, HD], f32r, isOutput=False)
    out = nc.declare_dram_parameter("o", [D, JQ], f32, isOutput=True)

    from contextlib import ExitStack

    with tile.TileContext(nc) as tc, ExitStack() as ctx, \
            nc.allow_low_precision("fp32r matmul inputs keep ~19 bits"):
        ec = ctx.enter_context
        consts = ec(tc.tile_pool(name="consts", bufs=1))
        big = ec(tc.tile_pool(name="big", bufs=1))
        wq_p = ec(tc.tile_pool(name="wq", bufs=3))
        xq_p = ec(tc.tile_pool(name="xq", bufs=1))
        xt_p = ec(tc.tile_pool(name="xt", bufs=2))
        wkv_p = ec(tc.tile_pool(name="wkv", bufs=4))
        vst_p = ec(tc.tile_pool(name="vst", bufs=2))
        vh_p = ec(tc.tile_pool(name="vh", bufs=2))
        e_p = ec(tc.tile_pool(name="e", bufs=6))
        ot_p = ec(tc.tile_pool(name="ot", bufs=2))
        rc_p = ec(tc.tile_pool(name="rc", bufs=2))
        wo_p = ec(tc.tile_pool(name="wo", bufs=2))
        yt_p = ec(tc.tile_pool(name="yt", bufs=2))
        dram = ec(tc.tile_pool(name="vdram", bufs=1, space="DRAM"))
        proj_ps = ec(tc.tile_pool(name="proj_ps", bufs=3, space="PSUM"))
        st_ps = ec(tc.tile_pool(name="st_ps", bufs=2, space="PSUM"))
        ot_ps = ec(tc.tile_pool(name="ot_ps", bufs=2, space="PSUM"))
        bc_ps = ec(tc.tile_pool(name="bc_ps", bufs=1, space="PSUM"))

        # ---- constants into SBUF ----
        bq8_sb = consts.tile([128, MC], f32, tag="bq8")
        nc.sync.dma_start(out=bq8_sb, in_=bq8.rearrange("(c p) -> p c", p=128))
        bk_sb = consts.tile([128, MC], f32, tag="bk")
        nc.sync.dma_start(out=bk_sb, in_=bkv.rearrange("(c p) -> p c", p=128))
        ob_sb = consts.tile([128, MC], f32, tag="ob")
        nc.sync.dma_start(out=ob_sb, in_=obias.rearrange("(c p) -> p c", p=128))
        pmb_sb = consts.tile([128, NKT], f32, tag="pmb")
        nc.sync.dma_start(out=pmb_sb, in_=pmb.rearrange("(t p) -> p t", p=128))
        cm_sb = consts.tile([128, 8, JB_N], f32r, tag="cm")
        nc.sync.dma_start(out=cm_sb, in_=cmask.rearrange("t p j -> p t j"))
        ones_sb = consts.tile([1, HD], f32r, tag="ones")
        nc.sync.dma_start(out=ones_sb, in_=onesc[:, :])

        # persistent activations
        QT_sb = big.tile([128, MC, JQ], f32r, tag="qt")       # 16KB/part
        KT_sb = big.tile([128, MC, S], f32r, tag="kt")        # 64KB/part
        AT_sb = big.tile([128, MC, JQ], f32r, tag="at")       # 16KB/part
        # V bounce in DRAM: [head, part(k%128), ktile, 65]
        vdram = dram.tile([H, 128, NKT, HD + 1], f32r, tag="v")

        # ---- Q projection: QT[dh, jq] = Wq x^T /8 + bq/8 ----
        xq_sb = xq_p.tile([128, MC, JQ], f32r, tag="xq")
        nc.sync.dma_start(out=xq_sb,
                          in_=xqT.rearrange("(c p) j -> p c j", p=128))
        for dt_ in range(MC):
            ps = proj_ps.tile([128, JQ], f32)
            for m in range(MC):
                w = wq_p.tile([128, 128], f32r, tag="wq")
                nc.sync.dma_start(
                    out=w, in_=wqT[m * 128:(m + 1) * 128,
                                   dt_ * 128:(dt_ + 1) * 128])
                nc.tensor.matmul(ps[:], w[:], xq_sb[:, m, :],
                                 start=(m == 0), stop=(m == MC - 1))
            nc.scalar.activation(out=QT_sb[:, dt_, :], in_=ps[:], func=IDENT,
                                 bias=bq8_sb[:, dt_:dt_ + 1], scale=0.125)

        # ---- K/V projections, streamed by 512-wide key block ----
        for kb in range(4):
            xt_sb = xt_p.tile([128, MC, 512], f32r, tag="xt")
            nc.sync.dma_start(
                out=xt_sb,
                in_=xT.rearrange("(c p) k -> p c k", p=128)[:, :, kb * 512:(kb + 1) * 512])
            # KT
            for dt_ in range(MC):
                ps = proj_ps.tile([128, 512], f32)
                for m in range(MC):
                    w = wkv_p.tile([128, 128], f32r, tag="wk")
                    nc.sync.dma_start(
                        out=w, in_=wkT[m * 128:(m + 1) * 128,
                                       dt_ * 128:(dt_ + 1) * 128])
                    nc.tensor.matmul(ps[:], w[:], xt_sb[:, m, :],
                                     start=(m == 0), stop=(m == MC - 1))
                nc.scalar.activation(
                    out=KT_sb[:, dt_, kb * 512:(kb + 1) * 512], in_=ps[:],
                    func=IDENT, bias=bk_sb[:, dt_:dt_ + 1])
            # V for the 4 key tiles in this block
            for kt_i in range(4):
                kt = kb * 4 + kt_i
                for half in range(2):
                    ps = proj_ps.tile([128, 512], f32)
                    for m in range(MC):
                        w = wkv_p.tile([128, 512], f32r, tag="wv")
                        nc.sync.dma_start(
                            out=w, in_=wvT[m * 128:(m + 1) * 128,
                                           half * 512:(half + 1) * 512])
                        nc.tensor.matmul(
                            ps[:], xt_sb[:, m, kt_i * 128:(kt_i + 1) * 128],
                            w[:], start=(m == 0), stop=(m == MC - 1))
                    vs = vst_p.tile([128, 8, HD], f32r, tag="vst")
                    nc.scalar.activation(
                        out=vs[:], in_=ps[:].rearrange("p (h d) -> p h d", d=HD),
                        func=COPY)
                    nc.sync.dma_start(
                        out=vdram[half * 8:(half + 1) * 8, :, kt, 0:HD]
                        .rearrange("h p d -> p h d"),
                        in_=vs[:])
        # ones column of V_aug (softmax denominator accumulator)
        nc.sync.dma_start(
            out=vdram.rearrange("h p t d -> (h p t) d")[:, HD:HD + 1],
            in_=onesc[0:1, 0:1].to_broadcast([H * 128 * NKT, 1]))

        # ---- attention per head ----
        for h in range(H):
            pr, hw = h // 2, 64 * (h % 2)
            vh = vh_p.tile([128, NKT, HD + 1], f32r, tag="vh")
            nc.sync.dma_start(out=vh, in_=vdram[h].rearrange("p t d -> p t d"))
            for jb in range(2):
                nkt = 8 if jb == 0 else 16
                otp = ot_ps.tile([HD + 1, JB_N], f32)
                for kt in range(nkt):
                    st = st_ps.tile([128, JB_N], f32)
                    nc.tensor.matmul(
                        st[:],
                        KT_sb[hw:hw + 64, pr, kt * 128:(kt + 1) * 128],
                        QT_sb[hw:hw + 64, pr, jb * JB_N:(jb + 1) * JB_N],
                        start=True, stop=True)
                    e = e_p.tile([128, JB_N], f32r, tag="e")
                    nc.scalar.activation(out=e[:], in_=st[:], func=EXP,
                                         bias=pmb_sb[:, kt:kt + 1])
                    tp = kt - 8 * jb
                    if tp >= 0:
                        nc.vector.tensor_mul(e[:], e[:], cm_sb[:, tp, :])
                    nc.tensor.matmul(otp[:], vh[:, kt, :], e[:],
                                     start=(kt == 0), stop=(kt == nkt - 1))
                ot_sb = ot_p.tile([HD + 1, JB_N], f32, tag="ot")
                nc.scalar.activation(out=ot_sb[:], in_=otp[:], func=COPY)
                rc = rc_p.tile([1, JB_N], f32r, tag="rc")
                nc.vector.reciprocal(out=rc[:], in_=ot_sb[HD:HD + 1, :])
                bc = bc_ps.tile([HD, JB_N], f32)
                nc.tensor.matmul(bc[:], ones_sb[:], rc[:], start=True, stop=True)
                nc.vector.tensor_mul(
                    AT_sb[hw:hw + 64, pr, jb * JB_N:(jb + 1) * JB_N],
                    ot_sb[0:HD, :], bc[:])

        # ---- output projection: YT[n, jq] ----
        for nt in range(MC):
            wo_sb = wo_p.tile([128, MC, 128], f32r, tag="wo")
            nc.sync.dma_start(
                out=wo_sb,
                in_=woT.rearrange("(c p) n -> p c n", p=128)[:, :, nt * 128:(nt + 1) * 128])
            ps = proj_ps.tile([128, JQ], f32)
            for c in range(MC):
                nc.tensor.matmul(ps[:], wo_sb[:, c, :], AT_sb[:, c, :],
                                 start=(c == 0), stop=(c == MC - 1))
            yt = yt_p.tile([128, JQ], f32, tag="yt")
            nc.scalar.activation(out=yt[:], in_=ps[:], func=IDENT,
                                 bias=ob_sb[:, nt:nt + 1])
            nc.sync.dma_start(out=out[nt * 128:(nt + 1) * 128, :], in_=yt[:])

    _split_waits(nc, mybir)
    return nc


def _get_nc():
    if "nc" not in _CACHE:
        _CACHE["nc"] = _build()
    return _CACHE["nc"]


def _make_inputs(x, mask, Wq, bq, Wk, bk, Wv, bv, Wo, bo):
    f = np.float32
    x = np.asarray(x, f)
    mask = np.asarray(mask)
    Wq, bq = np.asarray(Wq, f), np.asarray(bq, f)
    Wk, bk = np.asarray(Wk, f), np.asarray(bk, f)
    Wv, bv = np.asarray(Wv, f), np.asarray(bv, f)
    Wo, bo = np.asarray(Wo, f), np.asarray(bo, f)

    wqT = np.ascontiguousarray(Wq.T)
    wkT = np.ascontiguousarray(Wk.T)
    wvT = np.ascontiguousarray(Wv.T)
    woT = np.ascontiguousarray(Wo.T)
    bq8 = (bq / 8.0).astype(f)
    obias = (bo + Wo @ bv).astype(f)

    xTb = [np.ascontiguousarray(x[b].T) for b in range(B)]
    pmbb = [((mask[b].astype(f) - 1.0) * 1e4).astype(f) for b in range(B)]

    ii, jj = np.meshgrid(np.arange(128), np.arange(JB_N), indexing="ij")
    onesc = np.ones((1, HD), f)

    ins = []
    for c in range(N_CORES):
        b, r = c // 4, c % 4
        cm = np.empty((8, 128, JB_N), f)
        for tp in range(8):
            cm[tp] = (128 * tp + ii <= 4 * jj + r).astype(f)
        ins.append({
            "xT": xTb[b],
            "xqT": np.ascontiguousarray(x[b].T[:, r::4]),
            "wqT": wqT, "wkT": wkT, "wvT": wvT, "woT": woT,
            "bq8": bq8, "bk": bk, "obias": obias,
            "pmb": pmbb[b],
            "cmask": cm,
            "onesc": onesc,
        })
    return ins


def _run(ins, trace=False):
    from concourse.bass_utils import run_bass_kernel_spmd
    nc = _get_nc()
    return run_bass_kernel_spmd(nc, ins, list(range(N_CORES)), trace=trace)


def kernel(x, mask, Wq, bq, Wk, bk, Wv, bv, Wo, bo):
    ins = _make_inputs(x, mask, Wq, bq, Wk, bk, Wv, bv, Wo, bo)
    res = _run(ins)
    out = np.empty((B, S, D), np.float32)
    for c in range(N_CORES):
        b, r = c // 4, c % 4
        out[b, r::4, :] = res.results[c]["o"].T
    return out


# revision 12
# speedup vs baseline: 1.8745x; 1.8745x over previous
"""Multi-head self-attention (B=2, S=2048, D=1024, H=16, causal+padding mask)
on 8 Trainium2 NeuronCores via Bass/Tile, SPMD.

Sharding: core c -> batch b = c//4, query residue r = c%4. Each core computes
the full K/V projections for its batch (duplicated across the 4 cores of a
batch -- cheaper than any cross-core collective at this size) and attention +
output projection for the strided query set q = 4j + r, j = 0..511. Strided
(rather than blocked) query assignment keeps the causal loop structure
identical on every core, which SPMD requires; per-core differences (mask
values, packed activations) travel as data.

Dataflow is fully transposed so no on-chip transposes are needed:
  QT[dh, q]   = (Wq x^T)/8 + bq/8          lhsT = Wq^T chunks, rhs = xq^T
  KT[dh, k]   = Wk x^T + bk
  V [k, dh]   = x Wv^T  (+ ones column)    lhsT = x^T chunks,  rhs = Wv^T
  ST[k, q]    = KT_h^T QT_h  (per head)
  E           = exp(ST + padmask_bias) * causal01
  OT'[dh+1,q] = V_aug^T E   (row 64 = softmax denominators r[q])
  AT[dh, q]   = OT' * (1/r)                (broadcast r via rank-1 matmul,
                                            then reciprocal on all 64 lanes)
  YT[n, q]    = Wo_arr^T AT + (bo + Wo bv) lhsT = Wo^T chunks, rhs = AT
Matmuls run in float32r (full-rate fp32 with reduced mantissa, ~1e-4 rel
error). Softmax skips max-subtraction: scores are bounded (|S| < ~5) so exp
is safe, and masked lanes get -1e4 added pre-exp which underflows to exactly
0 after exp.
"""

import sys

if "/opt/trn_rl_repo" not in sys.path:
    sys.path.insert(0, "/opt/trn_rl_repo")

import numpy as np

B, S, D, H, HD = 2, 2048, 1024, 16, 64
N_CORES = 8
JQ = S // 4          # 512 packed queries per core
MC = D // 128        # 8 contraction chunks of 128
NKT = S // 128       # 16 key tiles
JB_N = 256           # packed query block (matmul N)

_CACHE = {}


def _split_waits(nc, mybir):
    """This walrus build accepts only one sync-wait per instruction; move
    extra waits onto NOPs inserted just before, on the same engine."""
    n_new = 0
    for f in nc.m.functions:
        for blk in f.blocks:
            out = []
            for inst in blk.instructions:
                si = inst.sync_info
                if si is not None and si.on_wait is not None and len(si.on_wait) > 1:
                    waits = list(si.on_wait)
                    for w in waits[:-1]:
                        n_new += 1
                        out.append(mybir.InstNoOp(
                            name=f"I-waitsplit-{n_new}",
                            engine=inst.engine,
                            ins=[], outs=[],
                            sync_info=mybir.SyncInfo(on_wait=[w], on_update=[]),
                        ))
                    inst.sync_info = mybir.SyncInfo(
                        on_wait=[waits[-1]], on_update=list(si.on_update or []))
                out.append(inst)
            blk.instructions[:] = out
    return n_new


def _build():
    import concourse.bass as bass
    import concourse.mybir as mybir
    import concourse.tile as tile
    from contextlib import ExitStack

    f32 = mybir.dt.float32
    f32r = mybir.dt.float32r
    EXP = mybir.ActivationFunctionType.Exp
    IDENT = mybir.ActivationFunctionType.Identity
    COPY = mybir.ActivationFunctionType.Copy

    nc = bass.Bass()
    xT = nc.declare_dram_parameter("xT", [D, S], f32r, isOutput=False)
    xqT = nc.declare_dram_parameter("xqT", [D, JQ], f32r, isOutput=False)
    wqT = nc.declare_dram_parameter("wqT", [D, D], f32r, isOutput=False)
    wkT = nc.declare_dram_parameter("wkT", [D, D], f32r, isOutput=False)
    wvT = nc.declare_dram_parameter("wvT", [D, D], f32r, isOutput=False)
    woT = nc.declare_dram_parameter("woT", [D, D], f32r, isOutput=False)
    bq8 = nc.declare_dram_parameter("bq8", [D], f32, isOutput=False)
    bkv = nc.declare_dram_parameter("bk", [D], f32, isOutput=False)
    obias = nc.declare_dram_parameter("obias", [D], f32, isOutput=False)
    pmb = nc.declare_dram_parameter("pmb", [S], f32, isOutput=False)
    cmask = nc.declare_dram_parameter("cmask", [8, 128, JB_N], f32r, isOutput=False)
    onesc = nc.declare_dram_parameter("onesc", [1, HD], f32r, isOutput=False)
    vones = nc.declare_dram_parameter("vones", [128, NKT * H], f32r, isOutput=False)
    out = nc.declare_dram_parameter("o", [D, JQ], f32, isOutput=True)

    with tile.TileContext(nc) as tc, ExitStack() as ctx, \
            nc.allow_low_precision("fp32r matmul inputs keep ~19 bits"):
        ec = ctx.enter_context
        consts = ec(tc.tile_pool(name="consts", bufs=1))
        big = ec(tc.tile_pool(name="big", bufs=1))
        e_p = ec(tc.tile_pool(name="e", bufs=4))
        rc_p = ec(tc.tile_pool(name="rc", bufs=1))
        rb_p = ec(tc.tile_pool(name="rb", bufs=1))
        yt_p = ec(tc.tile_pool(name="yt", bufs=2))
        proj_ps = ec(tc.tile_pool(name="proj_ps", bufs=3, space="PSUM"))
        st_ps = ec(tc.tile_pool(name="st_ps", bufs=2, space="PSUM"))
        ot_ps = ec(tc.tile_pool(name="ot_ps", bufs=2, space="PSUM"))
        bc_ps = ec(tc.tile_pool(name="bc_ps", bufs=1, space="PSUM"))

        # ---- constants into SBUF ----
        bq8_sb = consts.tile([128, MC], f32, tag="bq8")
        nc.sync.dma_start(out=bq8_sb, in_=bq8.rearrange("(c p) -> p c", p=128))
        bk_sb = consts.tile([128, MC], f32, tag="bk")
        nc.sync.dma_start(out=bk_sb, in_=bkv.rearrange("(c p) -> p c", p=128))
        ob_sb = consts.tile([128, MC], f32, tag="ob")
        nc.sync.dma_start(out=ob_sb, in_=obias.rearrange("(c p) -> p c", p=128))
        pmb_sb = consts.tile([128, NKT], f32, tag="pmb")
        nc.sync.dma_start(out=pmb_sb, in_=pmb.rearrange("(t p) -> p t", p=128))
        cm_sb = consts.tile([128, 8, JB_N], f32r, tag="cm")
        nc.sync.dma_start(out=cm_sb, in_=cmask.rearrange("t p j -> p t j"))
        ones_sb = consts.tile([1, HD], f32r, tag="ones")
        nc.sync.dma_start(out=ones_sb, in_=onesc[:, :])

        # persistent activations
        QT_sb = big.tile([128, MC, JQ], f32r, tag="qt")            # 16KB/part
        KT_sb = big.tile([128, MC, S], f32r, tag="kt")             # 64KB/part
        V_sb = big.tile([128, NKT, H, HD + 1], f32r, tag="v")      # 66.6KB/part
        # xq (Q-proj phase) and AT (attention/output phases) have disjoint
        # lifetimes; share one 16KB slot via a common tag.
        xq_sb = big.tile([128, MC, JQ], f32r, tag="xqat")
        # softmax-denominator ones column of V_aug
        nc.sync.dma_start(
            out=V_sb.rearrange("p k h d -> p (k h) d")[:, :, HD:HD + 1],
            in_=vones.rearrange("p (n o) -> p n o", o=1))

        # ---- Q projection: QT[dh, jq] = Wq x^T /8 + bq/8 ----
        nc.sync.dma_start(out=xq_sb,
                          in_=xqT.rearrange("(c p) j -> p c j", p=128))
        with tc.tile_pool(name="wq", bufs=1) as wq_p:
            for half in range(2):
                wq_sb = wq_p.tile([128, MC, 512], f32r, tag="wq")
                nc.sync.dma_start(
                    out=wq_sb,
                    in_=wqT.rearrange("(c p) n -> p c n", p=128)[:, :, half * 512:(half + 1) * 512])
                for dt_i in range(4):
                    dt_ = half * 4 + dt_i
                    ps = proj_ps.tile([128, 512], f32)
                    for m in range(MC):
                        nc.tensor.matmul(
                            ps[:], wq_sb[:, m, dt_i * 128:(dt_i + 1) * 128],
                            xq_sb[:, m, :],
                            start=(m == 0), stop=(m == MC - 1))
                    nc.scalar.activation(
                        out=QT_sb[:, dt_, :], in_=ps[:], func=IDENT,
                        bias=bq8_sb[:, dt_:dt_ + 1], scale=0.125)

        # ---- K projection: x^T streamed by 512-key block, Wk in 256-wide
        # column chunks (keeps phase SBUF <= 24KB/partition) ----
        with tc.tile_pool(name="kproj", bufs=1) as kp:
            for kb in range(4):
                xt_sb = kp.tile([128, MC, 512], f32r, tag="xt")
                nc.sync.dma_start(
                    out=xt_sb,
                    in_=xT.rearrange("(c p) k -> p c k", p=128)[:, :, kb * 512:(kb + 1) * 512])
                for q4 in range(4):
                    wk_sb = kp.tile([128, MC, 256], f32r, tag="wk")
                    nc.sync.dma_start(
                        out=wk_sb,
                        in_=wkT.rearrange("(c p) n -> p c n", p=128)[:, :, q4 * 256:(q4 + 1) * 256])
                    for dt_i in range(2):
                        dt_ = q4 * 2 + dt_i
                        ps = proj_ps.tile([128, 512], f32)
                        for m in range(MC):
                            nc.tensor.matmul(
                                ps[:], wk_sb[:, m, dt_i * 128:(dt_i + 1) * 128],
                                xt_sb[:, m, :],
                                start=(m == 0), stop=(m == MC - 1))
                        nc.scalar.activation(
                            out=KT_sb[:, dt_, kb * 512:(kb + 1) * 512], in_=ps[:],
                            func=IDENT, bias=bk_sb[:, dt_:dt_ + 1])

        # ---- V projection: Wv dh-half resident, x^T streamed in 256-key
        # blocks (2 key tiles each) ----
        with tc.tile_pool(name="vproj", bufs=1) as vp:
            for half in range(2):
                wv_sb = vp.tile([128, MC, 512], f32r, tag="wv")
                nc.sync.dma_start(
                    out=wv_sb,
                    in_=wvT.rearrange("(c p) n -> p c n", p=128)[:, :, half * 512:(half + 1) * 512])
                for kb8 in range(8):
                    xt_sb = vp.tile([128, MC, 256], f32r, tag="xtv")
                    nc.sync.dma_start(
                        out=xt_sb,
                        in_=xT.rearrange("(c p) k -> p c k", p=128)[:, :, kb8 * 256:(kb8 + 1) * 256])
                    for kt_i in range(2):
                        kt = kb8 * 2 + kt_i
                        ps = proj_ps.tile([128, 512], f32)
                        for m in range(MC):
                            nc.tensor.matmul(
                                ps[:], xt_sb[:, m, kt_i * 128:(kt_i + 1) * 128],
                                wv_sb[:, m, :],
                                start=(m == 0), stop=(m == MC - 1))
                        nc.scalar.activation(
                            out=V_sb[:, kt, half * 8:(half + 1) * 8, 0:HD],
                            in_=ps[:].rearrange("p (h d) -> p h d", d=HD),
                            func=COPY)

        # ---- attention per head ----
        AT_sb = big.tile([128, MC, JQ], f32r, tag="xqat")
        for h in range(H):
            pr, hw = h // 2, 64 * (h % 2)
            for jb in range(2):
                nkt = 8 if jb == 0 else 16
                otp = ot_ps.tile([HD + 1, JB_N], f32)
                for kt in range(nkt):
                    st = st_ps.tile([128, JB_N], f32)
                    nc.tensor.matmul(
                        st[:],
                        KT_sb[hw:hw + 64, pr, kt * 128:(kt + 1) * 128],
                        QT_sb[hw:hw + 64, pr, jb * JB_N:(jb + 1) * JB_N],
                        start=True, stop=True)
                    e = e_p.tile([128, JB_N], f32r, tag="e")
                    nc.scalar.activation(out=e[:], in_=st[:], func=EXP,
                                         bias=pmb_sb[:, kt:kt + 1])
                    tp = kt - 8 * jb
                    if tp >= 0:
                        nc.vector.tensor_mul(e[:], e[:], cm_sb[:, tp, :])
                    nc.tensor.matmul(otp[:], V_sb[:, kt, h, :], e[:],
                                     start=(kt == 0), stop=(kt == nkt - 1))
                # normalization: r row -> rank-1 broadcast -> reciprocal -> mul
                rc = rc_p.tile([1, JB_N], f32r, tag="rc")
                nc.scalar.activation(out=rc[:], in_=otp[HD:HD + 1, :], func=COPY)
                bc = bc_ps.tile([HD, JB_N], f32)
                nc.tensor.matmul(bc[:], ones_sb[:], rc[:], start=True, stop=True)
                rb = rb_p.tile([HD, JB_N], f32, tag="rb")
                nc.vector.reciprocal(out=rb[:], in_=bc[:])
                nc.vector.tensor_mul(
                    AT_sb[hw:hw + 64, pr, jb * JB_N:(jb + 1) * JB_N],
                    otp[0:HD, :], rb[:])

        # ---- output projection: YT[n, jq] ----
        with tc.tile_pool(name="oproj", bufs=1) as op:
            for half in range(2):
                wo_sb = op.tile([128, MC, 512], f32r, tag="wo")
                nc.sync.dma_start(
                    out=wo_sb,
                    in_=woT.rearrange("(c p) n -> p c n", p=128)[:, :, half * 512:(half + 1) * 512])
                for nt_i in range(4):
                    nt = half * 4 + nt_i
                    ps = proj_ps.tile([128, JQ], f32)
                    for c in range(MC):
                        nc.tensor.matmul(
                            ps[:], wo_sb[:, c, nt_i * 128:(nt_i + 1) * 128],
                            AT_sb[:, c, :],
                            start=(c == 0), stop=(c == MC - 1))
                    yt = yt_p.tile([128, JQ], f32, tag="yt")
                    nc.scalar.activation(out=yt[:], in_=ps[:], func=IDENT,
                                         bias=ob_sb[:, nt:nt + 1])
                    nc.sync.dma_start(out=out[nt * 128:(nt + 1) * 128, :], in_=yt[:])

    _split_waits(nc, mybir)
    return nc


def _get_nc():
    if "nc" not in _CACHE:
        _CACHE["nc"] = _build()
    return _CACHE["nc"]


def _make_inputs(x, mask, Wq, bq, Wk, bk, Wv, bv, Wo, bo):
    f = np.float32
    x = np.asarray(x, f)
    mask = np.asarray(mask)
    Wq, bq = np.asarray(Wq, f), np.asarray(bq, f)
    Wk, bk = np.asarray(Wk, f), np.asarray(bk, f)
    Wv, bv = np.asarray(Wv, f), np.asarray(bv, f)
    Wo, bo = np.asarray(Wo, f), np.asarray(bo, f)

    wqT = np.ascontiguousarray(Wq.T)
    wkT = np.ascontiguousarray(Wk.T)
    wvT = np.ascontiguousarray(Wv.T)
    woT = np.ascontiguousarray(Wo.T)
    bq8 = (bq / 8.0).astype(f)
    obias = (bo + Wo @ bv).astype(f)

    xTb = [np.ascontiguousarray(x[b].T) for b in range(B)]
    pmbb = [((mask[b].astype(f) - 1.0) * 1e4).astype(f) for b in range(B)]

    ii, jj = np.meshgrid(np.arange(128), np.arange(JB_N), indexing="ij")
    onesc = np.ones((1, HD), f)
    vones = np.ones((128, NKT * H), f)

    ins = []
    for c in range(N_CORES):
        b, r = c // 4, c % 4
        cm = np.empty((8, 128, JB_N), f)
        for tp in range(8):
            cm[tp] = (128 * tp + ii <= 4 * jj + r).astype(f)
        ins.append({
            "xT": xTb[b],
            "xqT": np.ascontiguousarray(x[b].T[:, r::4]),
            "wqT": wqT, "wkT": wkT, "wvT": wvT, "woT": woT,
            "bq8": bq8, "bk": bk, "obias": obias,
            "pmb": pmbb[b],
            "cmask": cm,
            "onesc": onesc,
            "vones": vones,
        })
    return ins


def _run(ins, trace=False):
    from concourse.bass_utils import run_bass_kernel_spmd
    nc = _get_nc()
    return run_bass_kernel_spmd(nc, ins, list(range(N_CORES)), trace=trace)


def kernel(x, mask, Wq, bq, Wk, bk, Wv, bv, Wo, bo):
    ins = _make_inputs(x, mask, Wq, bq, Wk, bk, Wv, bv, Wo, bo)
    res = _run(ins)
    out = np.empty((B, S, D), np.float32)
    for c in range(N_CORES):
        b, r = c // 4, c % 4
        out[b, r::4, :] = res.results[c]["o"].T
    return out
